# revision 1
# baseline (speedup 1.0000x reference)
"""Causal self-attention Trainium2 kernel (8 NeuronCores, SPMD).

Problem (hardcoded): x [4, 2048, 2048] f32, W_qkv [6144, 2048], W_out [2048, 2048],
16 heads x 128 dim, causal softmax attention + output projection.

Sharding: core c = 2*b + g handles batch b (4) and head-group g (2 groups of 8
heads).  Each core computes its 8 heads' QKV projection, attention, and the
partial output projection against its slice of W_out columns; the host sums the
two partials per batch element.

All matmuls run as float32r (full-rate fp32 on the PE array).  Per-head
attention computes S^T = K.Q^T tiles (k on partitions, q on the free dim) so
softmax renormalization sums arrive via a ones-vector matmul and the AV matmul
(lhsT = V) directly produces O^T, which feeds the output projection as lhsT.
Causal masking skips strictly-upper k-blocks and masks diagonal blocks.
"""

import math

import numpy as np

B = 4
T = 2048
C = 2048
H = 16          # total heads
HG = 8          # heads per core (tensor-parallel group)
D = 128         # head dim
P = 128         # partitions
NCS = C // P    # 16 contraction subtiles
NTC = T // P    # 16 T chunks of 128
NTB = T // 512  # 4 T blocks of 512
SCALE = 1.0 / math.sqrt(D)

_CACHED = None


def _build():
    import concourse.mybir as mybir
    from concourse import bacc
    from concourse.tile import TileContext

    f32 = mybir.dt.float32
    f32r = mybir.dt.float32r
    EXP = mybir.ActivationFunctionType.Exp
    MULT = mybir.AluOpType.mult

    nc = bacc.Bacc("TRN2", target_bir_lowering=False)

    xt_d = nc.dram_tensor("xt", [NCS, P, T], f32r, kind="ExternalInput")
    wq_d = nc.dram_tensor("wq", [HG, P, NCS, D], f32r, kind="ExternalInput")
    wk_d = nc.dram_tensor("wk", [HG, P, NCS, D], f32r, kind="ExternalInput")
    wv_d = nc.dram_tensor("wv", [P, NCS, HG * D], f32r, kind="ExternalInput")
    wo_d = nc.dram_tensor("wo", [HG * D, C], f32r, kind="ExternalInput")
    masks_d = nc.dram_tensor("masks", [4, P, 512], f32r, kind="ExternalInput")
    ones_d = nc.dram_tensor("ones", [P, 1], f32r, kind="ExternalInput")
    onesrow_d = nc.dram_tensor("ones_row", [1, P], f32, kind="ExternalInput")
    out_d = nc.dram_tensor("out", [T, C], f32, kind="ExternalOutput")

    with TileContext(nc) as tc:
        with tc.tile_pool(name="persist", bufs=1) as persist, \
             tc.tile_pool(name="dram", bufs=1, space="DRAM") as dram:
            masks_t = persist.tile([P, 4, 512], f32r, tag="masks")
            nc.sync.dma_start(masks_t, masks_d.rearrange("m p q -> p m q"))
            ones_t = persist.tile([P, 1], f32r, tag="ones")
            nc.sync.dma_start(ones_t, ones_d[:])
            onesrow_t = persist.tile([1, P], f32, tag="onesrow")
            nc.sync.dma_start(onesrow_t, onesrow_d[:])

            qt_dram = dram.tile([HG, P, T], f32r)    # Q^T per head
            kt_dram = dram.tile([HG, P, T], f32r)    # K^T per head
            v_dram = dram.tile([NTC, P, HG * D], f32r)  # V row-chunks, all heads

            # ---------------- Phase A: QKV projections ----------------
            with tc.tile_pool(name="xt", bufs=1) as xtp, \
                 tc.tile_pool(name="aw", bufs=2) as awp, \
                 tc.tile_pool(name="astage", bufs=4) as astage, \
                 tc.tile_pool(name="apsum", bufs=4, space="PSUM") as apsum:
                xt = []
                for cs in range(NCS):
                    t_ = xtp.tile([P, T], f32r, tag=f"xt{cs}")
                    nc.sync.dma_start(t_, xt_d[cs])
                    xt.append(t_)

                # Q^T and K^T: per head h, psum[128, 512] over T-blocks,
                # accumulating over the 16 C-subtiles.
                for w_d, dst in ((wq_d, qt_dram), (wk_d, kt_dram)):
                    for h in range(HG):
                        wt = awp.tile([P, NCS, D], f32r, tag="aw")
                        nc.sync.dma_start(wt, w_d[h])
                        for tb in range(NTB):
                            ps = apsum.tile([P, 512], f32, tag="aps")
                            for cs in range(NCS):
                                nc.tensor.matmul(
                                    ps, wt[:, cs], xt[cs][:, tb * 512:(tb + 1) * 512],
                                    start=(cs == 0), stop=(cs == NCS - 1))
                            st = astage.tile([P, 512], f32r, tag="ast")
                            nc.vector.tensor_copy(out=st, in_=ps)
                            nc.sync.dma_start(dst[h][:, tb * 512:(tb + 1) * 512], st)

                # V in [T, heads*D] layout: lhsT = xT chunk, rhs = W_v^T cols.
                for q in range(4):
                    wvq = awp.tile([P, NCS, 256], f32r, tag="awv")
                    nc.sync.dma_start(wvq, wv_d[:, :, q * 256:(q + 1) * 256])
                    for tch in range(NTC):
                        ps = apsum.tile([P, 256], f32, tag="apv")
                        for cs in range(NCS):
                            nc.tensor.matmul(
                                ps, xt[cs][:, tch * P:(tch + 1) * P], wvq[:, cs],
                                start=(cs == 0), stop=(cs == NCS - 1))
                        st = astage.tile([P, 256], f32r, tag="asv")
                        nc.vector.tensor_copy(out=st, in_=ps)
                        nc.sync.dma_start(
                            v_dram[tch][:, q * 256:(q + 1) * 256], st)

            # ---------------- Phase B: attention per head --------------
            with tc.tile_pool(name="ot", bufs=1) as otp:
                ot = []
                with tc.tile_pool(name="bhead", bufs=2) as bh, \
                     tc.tile_pool(name="bpt", bufs=4) as bptp, \
                     tc.tile_pool(name="bmisc", bufs=2) as bmisc, \
                     tc.tile_pool(name="bpsum", bufs=2, space="PSUM") as bps:
                    for h in range(HG):
                        qt_t = bh.tile([P, T], f32r, tag="qt")
                        nc.sync.dma_start(qt_t, qt_dram[h])
                        kt_t = bh.tile([P, T], f32r, tag="kt")
                        nc.sync.dma_start(kt_t, kt_dram[h])
                        v_t = bh.tile([P, NTC, D], f32r, tag="vh")
                        nc.sync.dma_start(
                            v_t, v_dram.rearrange("tc p hd -> p tc hd")[
                                :, :, h * D:(h + 1) * D])

                        ot_h = otp.tile([P, T], f32r, tag=f"ot{h}")
                        ot.append(ot_h)

                        for jb in range(NTB):   # q-blocks of 512
                            nk = 4 * (jb + 1)   # causal: k-chunks 0..nk-1
                            po = bps.tile([P, 512], f32, tag="po")
                            psum_s = bps.tile([P, 512], f32, tag="psu")
                            for ks in range(nk):
                                pst = bps.tile([P, 512], f32, tag="pst")
                                nc.tensor.matmul(
                                    pst, kt_t[:, ks * P:(ks + 1) * P],
                                    qt_t[:, jb * 512:(jb + 1) * 512],
                                    start=True, stop=True)
                                pt = bptp.tile([P, 512], f32r, tag="pt")
                                nc.scalar.activation(pt, pst, EXP, scale=SCALE)
                                m = ks - 4 * jb
                                if m >= 0:  # diagonal block: causal mask
                                    nc.vector.tensor_tensor(
                                        pt, pt, masks_t[:, m], MULT)
                                nc.tensor.matmul(
                                    po, v_t[:, ks], pt,
                                    start=(ks == 0), stop=(ks == nk - 1))
                                nc.tensor.matmul(
                                    psum_s[0:1], ones_t, pt,
                                    start=(ks == 0), stop=(ks == nk - 1))
                            # normalize: recip of row sums, broadcast via PE
                            recip = bmisc.tile([1, 512], f32, tag="rc")
                            nc.vector.reciprocal(recip, psum_s[0:1])
                            pb = bps.tile([P, 512], f32, tag="pb")
                            nc.tensor.matmul(pb, onesrow_t, recip,
                                             start=True, stop=True)
                            cpb = bmisc.tile([P, 512], f32, tag="cpb")
                            nc.scalar.copy(cpb, pb)
                            nc.vector.tensor_tensor(
                                ot_h[:, jb * 512:(jb + 1) * 512], po, cpb, MULT)

                # ------------- Phase C: output projection --------------
                with tc.tile_pool(name="cw", bufs=2) as cwp, \
                     tc.tile_pool(name="cstage", bufs=4) as cstage, \
                     tc.tile_pool(name="cpsum", bufs=4, space="PSUM") as cps:
                    wo_r = wo_d.rearrange("(h p) o -> p h o", p=P)
                    for ob in range(4):     # output blocks of 512
                        wo_t = cwp.tile([P, HG, 512], f32r, tag="cw")
                        nc.sync.dma_start(
                            wo_t, wo_r[:, :, ob * 512:(ob + 1) * 512])
                        for tch in range(NTC):
                            ps = cps.tile([P, 512], f32, tag="cps")
                            for h in range(HG):
                                nc.tensor.matmul(
                                    ps, ot[h][:, tch * P:(tch + 1) * P],
                                    wo_t[:, h],
                                    start=(h == 0), stop=(h == HG - 1))
                            st = cstage.tile([P, 512], f32, tag="cst")
                            nc.vector.tensor_copy(out=st, in_=ps)
                            nc.sync.dma_start(
                                out_d[tch * P:(tch + 1) * P,
                                      ob * 512:(ob + 1) * 512], st)

    nc.finalize()
    return nc


def _get_nc():
    global _CACHED
    if _CACHED is None:
        _CACHED = _build()
    return _CACHED


def _prep_inputs(x, W_qkv, W_out):
    """Host-side shard + layout prep. Returns per-core input maps."""
    f32 = np.float32
    x = np.asarray(x, dtype=f32)
    W_qkv = np.asarray(W_qkv, dtype=f32)
    W_out = np.asarray(W_out, dtype=f32)

    # causal masks for the 4 diagonal sub-positions of a 512-wide q block
    k_idx = np.arange(P)
    q_idx = np.arange(512)
    masks = np.stack([
        (q_idx[None, :] >= (m * P + k_idx)[:, None]).astype(f32)
        for m in range(4)
    ])  # [4, 128, 512]
    ones = np.ones((P, 1), dtype=f32)
    ones_row = np.ones((1, P), dtype=f32)

    per_g = {}
    for g in range(2):
        sl = slice(g * HG * D, (g + 1) * HG * D)
        wq = W_qkv[0 * C:1 * C][sl]        # [1024, 2048]
        wk = W_qkv[1 * C:2 * C][sl]
        wv = W_qkv[2 * C:3 * C][sl]
        # [h, p, cs, m]: element = w[h*128+m, cs*128+p]
        wq_a = np.ascontiguousarray(
            wq.reshape(HG, D, NCS, P).transpose(0, 3, 2, 1))
        wk_a = np.ascontiguousarray(
            wk.reshape(HG, D, NCS, P).transpose(0, 3, 2, 1))
        # [p, cs, hm]: element = wv[hm, cs*128+p]
        wv_a = np.ascontiguousarray(
            wv.reshape(HG * D, NCS, P).transpose(2, 1, 0))
        wo_a = np.ascontiguousarray(W_out[:, sl].T)   # [1024, 2048]
        per_g[g] = (wq_a, wk_a, wv_a, wo_a)

    in_maps = []
    for core in range(8):
        b, g = divmod(core, 2)
        xt = np.ascontiguousarray(x[b].T).reshape(NCS, P, T)
        wq_a, wk_a, wv_a, wo_a = per_g[g]
        in_maps.append({
            "xt": xt, "wq": wq_a, "wk": wk_a, "wv": wv_a, "wo": wo_a,
            "masks": masks, "ones": ones, "ones_row": ones_row,
        })
    return in_maps


def kernel(x, W_qkv, W_out, *, trace=False, trace_cores=None):
    from concourse.bass_utils import run_bass_kernel_spmd

    nc = _get_nc()
    in_maps = _prep_inputs(x, W_qkv, W_out)
    r = run_bass_kernel_spmd(
        nc, in_maps, core_ids=list(range(8)),
        trace=trace, trace_cores=trace_cores)

    out = np.empty((B, T, C), dtype=np.float32)
    for b in range(B):
        out[b] = r.results[2 * b]["out"] + r.results[2 * b + 1]["out"]
    if trace:
        kernel.last_results = r
    return out
